# revision 27
# baseline (speedup 1.0000x reference)
"""LocalGraphAttention TRN2 kernel v2: 8-core SPMD (batch x head-pair parallel).

Per core c: batch b = c//2, heads 4*(c%2)+[0..3]. All tensors transposed so
softmax key-reduction stays on the PE (S^T layout [keys, queries]).

Masked exp is split across three engine paths, assigned per key-block:
  A: ScalarE exp + VectorE 0/1-mask multiply           (S+V)
  B: VectorE fused Schraudolph exp2 bit-trick:
     int16(rint(s*A + B)) bits == bf16 exp(s); the mask multiply rides the
     same AFFINE_MUL_REDUCE op ((s*A+B)*m -> int16, masked -> +-0)   (V only)
  C: -30000 additive mask seeded into score PSUM by an identity matmul,
     then plain ScalarE exp (exp(-big) == 0)            (S+P)

Rowsum is fused into the PV matmul via a ones-column in the V weights
(v64 layout: [v_h | 1 | 0*31] per head, M=64 per head). Normalization uses
K=1 broadcast matmuls + reciprocal_approx_fast. bk bias is dropped (cancels
in softmax); bq folded into qt; bv folded into b_out on the host
(softmax weights sum to 1); out bias added on host.
"""
import sys
import numpy as np
import ml_dtypes

sys.path.insert(0, "/opt/trn_rl_repo")

from contextlib import ExitStack

import concourse.bass as bass
import concourse.mybir as mybir
import concourse.tile as tile
from concourse import bacc
from concourse.bass_utils import run_bass_kernel_spmd
from concourse.dve_ops import AFFINE_MUL_REDUCE

BF16 = ml_dtypes.bfloat16
G = 2048
D = 256
DH = 32
B = 4
NCORES = 8
SCALE = 1.0 / np.sqrt(np.float32(DH))
KB = G // 128   # 16 key blocks
QG = G // 512   # 4 query groups
NEG = -30000.0

# per-key-block mask form: C=additive (PE-seed + ScalarE exp),
# M=multiplicative (per-pair: A=ScalarE exp + DVE mask, B=DVE fused Schraudolph)
PATHS = "MCMCMCMCMCMCMCMC"
assert len(PATHS) == KB


def pair_path(qg, kb, p):
    if PATHS[kb] == "C":
        return "C"
    m = PATHS[:kb].count("M")
    return "A" if (2 * m + p) % 3 == 0 else "B"

LOG2E = 1.4426950408889634
A_CONST = 128.0 * LOG2E
B_CONST = 16250.375


def build_nc():
    nc = bacc.Bacc("TRN2", target_bir_lowering=False, debug=False)
    dt = mybir.dt
    xT = nc.declare_dram_parameter("xT", [D, G], dt.bfloat16, isOutput=False)
    Wq = nc.declare_dram_parameter("Wq", [D, 128], dt.bfloat16, isOutput=False)
    Wk = nc.declare_dram_parameter("Wk", [D, 128], dt.bfloat16, isOutput=False)
    Wv = nc.declare_dram_parameter("Wv", [D, 128], dt.bfloat16, isOutput=False)
    bq = nc.declare_dram_parameter("bq", [128, 1], dt.float32, isOutput=False)
    MSK = nc.declare_dram_parameter("MSK", [G, G], dt.bfloat16, isOutput=False)
    IDI = nc.declare_dram_parameter("IDI", [128, 128], dt.bfloat16, isOutput=False)
    WOE = nc.declare_dram_parameter("WOE", [128, 2 * D], dt.bfloat16, isOutput=False)
    OUT = nc.declare_dram_parameter("out", [D, G], dt.float32, isOutput=True)
    Exp = mybir.ActivationFunctionType.Exp

    with tile.TileContext(nc) as tc, ExitStack() as ctx:
        sing = ctx.enter_context(tc.tile_pool(name="sing", bufs=1))
        maskp = ctx.enter_context(tc.tile_pool(name="maskp", bufs=KB))
        work = ctx.enter_context(tc.tile_pool(name="work", bufs=9))
        recp = ctx.enter_context(tc.tile_pool(name="recp", bufs=8))
        rcpp = ctx.enter_context(tc.tile_pool(name="rcpp", bufs=2))
        otp = ctx.enter_context(tc.tile_pool(name="otp", bufs=2))
        psq = ctx.enter_context(tc.tile_pool(name="psq", bufs=3, space="PSUM"))
        ppv = ctx.enter_context(tc.tile_pool(name="ppv", bufs=1, space="PSUM"))

        # ---- ACT table warm-up (exp) before any real dependency ----
        wrm = sing.tile([128, 1], dt.bfloat16, tag="wrm")
        nc.vector.memset(wrm[:], 0.0)
        nc.scalar.activation(wrm[:], wrm[:], Exp, scale=1.0)

        # ---- resident loads (xT chunked so QKV can start early) ----
        xt = []
        for kc in range(2):
            t = sing.tile([128, G], dt.bfloat16, tag=f"xt{kc}")
            xt.append(t)
        for qg in range(QG):
            qsl = slice(512 * qg, 512 * (qg + 1))
            for kc in range(2):
                nc.sync.dma_start(out=xt[kc][:, qsl],
                                  in_=xT[128 * kc:128 * (kc + 1), qsl])
        wght = {}
        for name, p in (("wq", Wq), ("wk", Wk), ("wv", Wv)):
            for kc in range(2):
                t = sing.tile([128, 128], dt.bfloat16, tag=f"{name}{kc}")
                nc.sync.dma_start(out=t[:], in_=p[128 * kc:128 * (kc + 1), :])
                wght[f"{name}{kc}"] = t
        bq_sb = sing.tile([128, 1], dt.float32, tag="bq")
        nc.sync.dma_start(out=bq_sb[:], in_=bq[:])
        idi_sb = sing.tile([128, 128], dt.bfloat16, tag="idi")
        nc.sync.dma_start(out=idi_sb[:], in_=IDI[:])
        woe_sb = sing.tile([128, 2 * D], dt.bfloat16, tag="woe")
        nc.sync.dma_start(out=woe_sb[:], in_=WOE[:])
        m_sb = []
        for kb in range(KB):
            t = maskp.tile([128, G], dt.bfloat16, tag="mask")
            nc.sync.dma_start(out=t[:], in_=MSK[128 * kb:128 * (kb + 1), :])
            m_sb.append(t)
        # K=64 broadcast weights: row 0 -> rows 0:64, row 32 -> rows 64:128
        ind2 = sing.tile([64, 128], dt.bfloat16, tag="ind2")
        nc.vector.memset(ind2[:], 0.0)
        nc.vector.memset(ind2[0:1, 0:64], 1.0)
        nc.vector.memset(ind2[32:33, 64:128], 1.0)
        junk = sing.tile([128, 1], dt.float32, tag="junk")
        rec_tiles = []
        for i in range(2 * QG):
            rt = sing.tile([64, 512], dt.bfloat16, tag=f"rec{i}")
            nc.vector.memset(rt[:], 0.0)
            rec_tiles.append(rt)

        # ---- QKV projections ----
        qt_sb = sing.tile([128, G], dt.bfloat16, tag="qt")
        kt_sb = sing.tile([128, G], dt.bfloat16, tag="kt")
        for qg in range(QG):
            qsl = slice(512 * qg, 512 * (qg + 1))
            ps = psq.tile([128, 1024], dt.float32, tag="sq")
            nc.tensor.matmul(ps[:, 0:512], wght["wq0"][:], xt[0][:, qsl],
                             start=True, stop=False)
            nc.tensor.matmul(ps[:, 0:512], wght["wq1"][:], xt[1][:, qsl],
                             start=False, stop=True)
            nc.tensor.matmul(ps[:, 512:1024], wght["wk0"][:], xt[0][:, qsl],
                             start=True, stop=False)
            nc.tensor.matmul(ps[:, 512:1024], wght["wk1"][:], xt[1][:, qsl],
                             start=False, stop=True)
            nc.vector.tensor_scalar_add(qt_sb[:, qsl], ps[:, 0:512], bq_sb[:])
            nc.scalar.copy(kt_sb[:, qsl], ps[:, 512:1024])

        # v64: [v_h(32) | ones(1) | zeros(31)] x4 heads per kb, one big tile
        v64 = sing.tile([128, KB * 256], dt.bfloat16, tag="v64")
        nc.vector.memset(v64[:], 0.0)
        nc.vector.memset(
            v64[:].rearrange("p (a b) -> p a b", b=64)[:, :, 32:33], 1.0)
        for kb in range(KB):
            ksl = slice(128 * kb, 128 * (kb + 1))
            ps = psq.tile([128, 1024], dt.float32, tag="sq")
            nc.tensor.matmul(ps[:, 0:128], xt[0][:, ksl], wght["wv0"][:],
                             start=True, stop=False)
            nc.tensor.matmul(ps[:, 0:128], xt[1][:, ksl], wght["wv1"][:],
                             start=False, stop=True)
            dst = v64[:, 256 * kb:256 * (kb + 1)]
            nc.vector.tensor_copy(
                dst.rearrange("p (a b) -> p a b", b=64)[:, :, 0:32],
                ps[:, 0:128].rearrange("p (a b) -> p a b", b=32))

        # ---- attention: one flat software-pipelined stream over all tiles.
        # PV matmuls lag PV_LAG tiles behind their exp; each qg's normalize
        # and out-projection are deferred into the next qg's tile stream so
        # the PE queue never goes sparse (sparse phases re-throttle HAM).
        PV_LAG = 6
        ynA = sing.tile([128, G], dt.bfloat16, tag="ynA")
        ynB = sing.tile([128, G], dt.bfloat16, tag="ynB")

        def emit_pv(pvt, kb, p, em):
            for j in range(2):
                nc.tensor.matmul(
                    pvt[p][64 * j:64 * (j + 1), :],
                    v64[:, 256 * kb + 128 * p + 64 * j:
                        256 * kb + 128 * p + 64 * (j + 1)],
                    em[:, 512 * j:512 * (j + 1)],
                    start=(kb == 0), stop=(kb == KB - 1),
                    tile_position=(0, 64 * j),
                    skip_group_check=True)

        def make_norm_stages(qg, pvt):
            qsl = slice(512 * qg, 512 * (qg + 1))
            recs = []
            rcp_box = []

            def stage_rec():
                with nc.allow_low_precision("softmax rowsum bf16"):
                    for half in range(2):
                        rec = rec_tiles[2 * qg + half]
                        nc.vector.tensor_copy(rec[0:1, :],
                                              pvt[half][32:33, :])
                        nc.scalar.copy(rec[32:33, :], pvt[half][96:97, :])
                        recs.append(rec)

            def stage_bcast():
                bc = psq.tile([128, 1024], dt.float32, tag="sq")
                nc.tensor.matmul(bc[:, 0:512], ind2[:], recs[0][:],
                                 start=True, stop=True, skip_group_check=True)
                nc.tensor.matmul(bc[:, 512:1024], ind2[:], recs[1][:],
                                 start=True, stop=True, skip_group_check=True)
                rcp = rcpp.tile([128, 1024], dt.float32, tag="rcp")
                nc.vector.reciprocal_approx_fast(out=rcp[:], in_=bc[:])
                rcp_box.append(rcp)

            def stage_yn():
                rcp = rcp_box[0]
                nc.vector.tensor_mul(ynA[:, qsl], pvt[0][:], rcp[:, 0:512])
                nc.vector.tensor_mul(ynB[:, qsl], pvt[1][:], rcp[:, 512:1024])

            def stage_oproj():
                op = psq.tile([128, 1024], dt.float32, tag="sq")
                for mt in range(2):
                    osl = slice(512 * mt, 512 * (mt + 1))
                    nc.tensor.matmul(op[:, osl],
                                     woe_sb[:, 128 * mt:128 * (mt + 1)],
                                     ynA[:, qsl], start=True, stop=False)
                    nc.tensor.matmul(
                        op[:, osl],
                        woe_sb[:, 256 + 128 * mt:256 + 128 * (mt + 1)],
                        ynB[:, qsl], start=False, stop=True)
                ot = otp.tile([128, 1024], dt.float32, tag="ot")
                if qg % 2 == 0:
                    nc.vector.tensor_copy(ot[:], op[:])
                else:
                    nc.scalar.copy(ot[:], op[:])
                for mt in range(2):
                    nc.sync.dma_start(
                        out=OUT[128 * mt:128 * (mt + 1), qsl],
                        in_=ot[:, 512 * mt:512 * (mt + 1)])

            return [stage_rec, stage_bcast, stage_yn, stage_oproj]

        tiles = [(qg, kb, p) for qg in range(QG) for kb in range(KB)
                 for p in range(2)]
        pend = []          # pending PV emissions
        deferred = {}      # tile index -> [callables]
        pvt = None
        for t, (qg, kb, p) in enumerate(tiles):
            if kb == 0 and p == 0:
                pvA = ppv.tile([128, 512], dt.float32, tag="pvA")
                pvB = ppv.tile([128, 512], dt.float32, tag="pvB")
                pvt = [pvA, pvB]
            for fn in deferred.pop(t, ()):
                fn()
            qsl = slice(512 * qg, 512 * (qg + 1))
            ksl = slice(128 * kb, 128 * (kb + 1))
            path = pair_path(qg, kb, p)
            seeded = path in ("C", "Cp", "Cs")
            sq = psq.tile([128, 1024], dt.float32, tag="sq")
            hs = (2 * p, 2 * p + 1)
            if path == "C":
                nc.tensor.matmul(sq[:, 0:512], idi_sb[:], m_sb[kb][:, qsl],
                                 start=True, stop=False,
                                 skip_group_check=True)
                nc.tensor.matmul(sq[:, 512:1024], idi_sb[:],
                                 m_sb[kb][:, qsl],
                                 start=True, stop=False,
                                 skip_group_check=True)
            for j, h in enumerate(hs):
                hsl = slice(32 * h, 32 * (h + 1))
                nc.tensor.matmul(
                    sq[:, 512 * j:512 * (j + 1)],
                    kt_sb[hsl, ksl], qt_sb[hsl, qsl],
                    start=(not seeded), stop=True,
                    tile_position=(32 * h, 0),
                    skip_group_check=True)
            em = work.tile([128, 1024], dt.bfloat16, tag="em")
            if seeded:
                nc.scalar.activation(em[:], sq[:], Exp, scale=1.0)
            elif path == "A":
                e = work.tile([128, 1024], dt.bfloat16, tag="e")
                nc.scalar.activation(e[:], sq[:], Exp, scale=1.0)
                for j in range(2):
                    esl = slice(512 * j, 512 * (j + 1))
                    nc.vector.tensor_mul(em[:, esl], e[:, esl],
                                         m_sb[kb][:, qsl])
            else:  # B
                mb = m_sb[kb][:, qsl].unsqueeze(1).broadcast_to(
                    [128, 2, 512])
                nc.vector._custom_dve(
                    AFFINE_MUL_REDUCE,
                    out=em[:].bitcast(dt.int16).rearrange(
                        "p (a b) -> p a b", a=2),
                    in0=sq[:].rearrange("p (a b) -> p a b", a=2),
                    in1=mb, s0=A_CONST, s1=B_CONST,
                    accum_out=junk[:])
            pend.append((pvt, kb, p, em))
            if len(pend) > PV_LAG:
                emit_pv(*pend.pop(0))
            if kb == KB - 1 and p == 1:
                # drain PVs so normalize reads see the full accumulation,
                # then defer normalize + out-proj into the next qg's stream
                for args in pend:
                    emit_pv(*args)
                pend = []
                stages = make_norm_stages(qg, pvt)
                for k, fn in enumerate(stages):
                    deferred.setdefault(t + 1 + 2 * k, []).append(fn)
        for t in sorted(deferred):
            if t >= len(tiles):
                for fn in deferred[t]:
                    fn()
    nc.finalize()
    return nc


_NC_CACHE = None
LAST_IN_MAPS = None


def kernel(x, allow_mask_bool, W_qkv, b_qkv, W_out, b_out):
    global _NC_CACHE, LAST_IN_MAPS
    x = np.asarray(x, np.float32)
    allow = np.asarray(allow_mask_bool)
    W_qkv = np.asarray(W_qkv, np.float32)
    b_qkv = np.asarray(b_qkv, np.float32)
    W_out = np.asarray(W_out, np.float32)
    b_out = np.asarray(b_out, np.float32)

    # masks: key-block rows in S^T layout; form depends on engine path
    mT = np.ascontiguousarray(allow.T).astype(np.float32)   # [keys, queries]
    msk = np.empty((G, G), dtype=BF16)
    for kb in range(KB):
        blk = mT[128 * kb:128 * (kb + 1)]
        if PATHS[kb] == "C":
            msk[128 * kb:128 * (kb + 1)] = ((blk - 1.0) * (-NEG)).astype(BF16)
        else:
            msk[128 * kb:128 * (kb + 1)] = blk.astype(BF16)
    idi = np.eye(128, dtype=np.float32).astype(BF16)

    in_maps = []
    for c in range(NCORES):
        b = c // 2
        hs = [4 * (c % 2) + i for i in range(4)]
        qcols = np.concatenate([np.arange(32 * h, 32 * h + 32) for h in hs])
        # woeA rows: h0 at 0:32, h1 at 64:96 ; woeB: h2 at 0:32, h3 at 64:96
        woe = np.zeros((128, 2 * D), np.float32)
        for j, h in enumerate(hs):
            half = j // 2          # 0 -> woeA, 1 -> woeB
            pos = 64 * (j % 2)     # row offset inside half
            woe[pos:pos + 32, D * half:D * (half + 1)] = \
                W_out[qcols[32 * j:32 * j + 32], :]
        m = {
            "xT": np.ascontiguousarray(x[b].T).astype(BF16),
            "Wq": np.ascontiguousarray(W_qkv[:, qcols] * SCALE).astype(BF16),
            "Wk": np.ascontiguousarray(W_qkv[:, 256 + qcols]).astype(BF16),
            "Wv": np.ascontiguousarray(W_qkv[:, 512 + qcols]).astype(BF16),
            "bq": np.ascontiguousarray(
                (b_qkv[qcols] * SCALE)[:, None]).astype(np.float32),
            "MSK": msk,
            "IDI": idi,
            "WOE": woe.astype(BF16),
        }
        in_maps.append(m)

    LAST_IN_MAPS = in_maps
    if _NC_CACHE is None:
        _NC_CACHE = build_nc()
    res = run_bass_kernel_spmd(_NC_CACHE, in_maps, core_ids=list(range(NCORES)))
    out = np.zeros((B, G, D), np.float32)
    for c in range(NCORES):
        out[c // 2] += res.results[c]["out"].T
    b_out_eff = b_out + b_qkv[512:768] @ W_out
    out += b_out_eff[None, None, :]
    return out


if __name__ == "__main__":
    rng = np.random.default_rng(0)
    ins = {
        "x": rng.standard_normal((B, G, D), dtype=np.float32),
        "allow_mask_bool": rng.random((G, G)) < 0.5,
        "W_qkv": rng.standard_normal((D, 3 * D), dtype=np.float32) * 0.06,
        "b_qkv": rng.standard_normal(3 * D).astype(np.float32) * 0.06,
        "W_out": rng.standard_normal((D, D), dtype=np.float32) * 0.06,
        "b_out": rng.standard_normal(D).astype(np.float32) * 0.06,
    }
    ins["allow_mask_bool"] |= np.eye(G, dtype=bool)
    out = kernel(**ins)
    print("kernel ran, out shape", out.shape)
